# revision 23
# baseline (speedup 1.0000x reference)
"""Haar-DWT L1 loss (DWTLoss) on 8 trn2 NeuronCores.

Math: the 2D haar DWT is linear, so p_coeffs - t_coeffs = haar(pred - target).
For each 2x2 block of d = pred - target with rows (a b / c d) the four
(unnormalized) subband values are
    s1 = a+b+c+d, s2 = a+b-c-d, s3 = a-b+c-d, s4 = a-b-c+d
and the loss contribution of the block is 0.5*(|s1|+|s2|+|s3|+|s4|)
(the 0.5 is the haar 1/2 normalization).  Summed over everything and
divided by the subband size N_SUB, times LOSS_WEIGHT=1.

Engine split (per 1 MiB chunk of each input):
  - DMA: pred/target chunks land as [128 rows, 4*512] tiles (partition =
    image row, so vertical neighbors sit in adjacent partitions).
  - Pool (gpsimd): d = pred - target.
  - PE: psum = V.T @ d where V is a constant +-1 matrix pairing adjacent
    partitions: psum rows 0..63 = row-pair sums (u), 64..127 = row-pair
    diffs (v).  Exact in fp32 (weights are +-1, two terms per output).
  - DVE: s_add = psum[even cols] + psum[odd cols]  (= s1 | s2 stacked),
         s_sub = psum[even cols] - psum[odd cols]  (= s3 | s4 stacked).
  - ACT: activation(Abs) with accum_out -> per-partition sums of |s|.
Host: sum the per-core [128, 48] partials, divide by 2*N_SUB.

This walrus build allows only ONE embedded sync-wait per instruction, so
`_hoist_excess_waits` moves extra waits onto standalone EventSemaphore
instructions on the same engine stream (semantics preserved: the engine
executes them in order before the instruction).

Sharding: pure data parallel over the batch dim (4 images per core); the
host reduces the 8 tiny partial tiles (the "all-reduce" of the hint).
"""

import os

import numpy as np

import concourse.bass as bass
import concourse.mybir as mybir
from concourse.bass_utils import run_bass_kernel_spmd
from concourse.tile import TileContext

B, C, H, W = 32, 3, 512, 512
N_CORES = 8
B_LOC = B // N_CORES                        # batch shard per core
N_SUB = B * C * (H // 2) * (W // 2)         # elements per DWT subband
P = 128                                     # SBUF partitions
GROUPS = 4                                  # 128-row groups per chunk
FREE = GROUPS * W                           # 2048 f32 per partition per chunk
N_ITER = (B_LOC * C * H * W) // (P * FREE)  # 12 chunks per core

F32 = mybir.dt.float32
ALU = mybir.AluOpType


def _hoist_excess_waits(nc):
    """Walrus in this toolchain allows one embedded sync-wait per
    instruction.  Tile sometimes attaches 2-3 (cross-engine + self + DMA).
    Hoist all but the last wait onto standalone same-engine EventSemaphore
    instructions inserted immediately before the offender — the engine
    stream executes them in order, so the AND-of-waits semantics and every
    sem value are preserved.  HW-compile path only: the injected bare
    instructions lack CoreSim bookkeeping (use _build() output for sim)."""
    n = 0
    for f in nc.m.functions:
        for bb in f.blocks:
            out = []
            for ins in bb.instructions:
                si = getattr(ins, "sync_info", None)
                ow = list(si.on_wait) if (si is not None and si.on_wait) else []
                if len(ow) > 1 and not isinstance(ins, mybir.InstEventSemaphore):
                    for w in ow[:-1]:
                        ev = mybir.InstEventSemaphore(name=f"{ins.name}-hw{n}")
                        n += 1
                        ev.engine = ins.engine
                        ev.sync_info = mybir.SyncInfo(on_wait=[w], on_update=[])
                        out.append(ev)
                    ins.sync_info = mybir.SyncInfo(
                        on_wait=[ow[-1]], on_update=list(si.on_update or [])
                    )
                out.append(ins)
            if n:
                bb.instructions[:] = out
    return nc


def make_vmat() -> np.ndarray:
    """[128, 128] +-1 pairing matrix: psum[m] = d[2m] + d[2m+1] for m<64,
    psum[m] = d[2(m-64)] - d[2(m-64)+1] for m>=64."""
    v = np.zeros((P, P), dtype=np.float32)
    for m in range(64):
        v[2 * m, m] = 1.0
        v[2 * m + 1, m] = 1.0
        v[2 * m, 64 + m] = 1.0
        v[2 * m + 1, 64 + m] = -1.0
    return v


def _build(sizes=None, BS=1, io_bufs=2, wk_bufs=4, ps_bufs=8, act_split=False):
    nc = bass.Bass()
    pred = nc.dram_tensor("pred", [B_LOC, C, H, W], F32, kind="ExternalInput")
    targ = nc.dram_tensor("target", [B_LOC, C, H, W], F32, kind="ExternalInput")
    vmat = nc.dram_tensor("vmat", [P, P], F32, kind="ExternalInput")

    # group G = 128 consecutive image rows; partition p = row within group
    NG = N_ITER * GROUPS  # 48 groups total
    pf = pred[:].flatten().rearrange("(G p w) -> p G w", G=NG, p=P, w=W)
    tf = targ[:].flatten().rearrange("(G p w) -> p G w", G=NG, p=P, w=W)

    # DMA granularity: ramp up from small chunks (so compute starts ~2us in,
    # not after two 2 MiB transfers), 2 MiB chunks (8 groups) in the steady
    # state for best HBM efficiency, ramp down at the end so the
    # post-last-DMA compute tail is short.  Compute granularity: 2-group
    # (1024-col) blocks (to amortize per-op overhead) with 4 PSUM tiles in
    # flight; 1-group blocks for the first/last chunks.
    if sizes is None:
        sizes = [4] * 11 + [2, 1, 1]
    chunks = []
    g = 0
    for s in sizes:
        chunks.append((g, s))
        g += s
    assert g == NG, (g, NG)
    n_blocks = sum((ng + BS - 1) // BS for _, ng in chunks)

    n_acc = 2 * n_blocks if act_split else n_blocks
    out = nc.dram_tensor("partial", [P, n_acc], F32, kind="ExternalOutput")

    BLK = BS * W  # compute block width

    with TileContext(nc) as tc:
        with (
            tc.tile_pool(name="io", bufs=io_bufs) as io,
            tc.tile_pool(name="wk", bufs=wk_bufs) as wk,
            tc.tile_pool(name="ps", bufs=ps_bufs, space="PSUM") as ps,
            tc.tile_pool(name="cst", bufs=1) as cst,
        ):
            vt = cst.tile([P, P], F32)
            nc.sync.dma_start(vt[:], vmat[:])
            acc = cst.tile([P, n_acc], F32)

            bi = 0
            for g0, ng in chunks:
                free = ng * W
                pt = io.tile([P, 8 * W], F32, tag="pt")
                tt = io.tile([P, 8 * W], F32, tag="tt")
                nc.sync.dma_start(pt[:, :free], pf[:, g0 : g0 + ng, :])
                nc.sync.dma_start(tt[:, :free], tf[:, g0 : g0 + ng, :])

                # split this chunk into 2-group (or remainder) compute blocks
                k = 0
                while k < ng:
                    nb = min(BS, ng - k)
                    bw = nb * W
                    hb = bw // 2
                    blk = slice(W * k, W * k + bw)
                    d = wk.tile([P, BLK], F32, tag="d")
                    nc.gpsimd.tensor_tensor(
                        d[:, :bw], pt[:, blk], tt[:, blk], ALU.subtract
                    )

                    psum = ps.tile([P, BLK], F32, tag="psum")
                    for m in range(nb):
                        nc.tensor.matmul(
                            psum[:, W * m : W * (m + 1)],
                            vt[:],
                            d[:, W * m : W * (m + 1)],
                            start=True,
                            stop=True,
                        )

                    # TT may read only one input from PSUM: stage odd columns
                    # into SBUF, then combine with the even-column view.  sa
                    # and sb live in one tile so ACT can abs+accumulate both
                    # in a single op.
                    s2 = wk.tile([P, BLK], F32, tag="s2")
                    odd = wk.tile([P, BLK // 2], F32, tag="odd")
                    pv = psum[:, :bw].rearrange("p (k two) -> p k two", k=hb, two=2)
                    nc.vector.tensor_copy(odd[:, :hb], pv[:, :, 1])
                    nc.vector.tensor_tensor(
                        s2[:, :hb], pv[:, :, 0], odd[:, :hb], ALU.add
                    )
                    nc.vector.tensor_tensor(
                        s2[:, hb : 2 * hb], pv[:, :, 0], odd[:, :hb], ALU.subtract
                    )

                    scr = wk.tile([P, BLK], F32, tag="scr")
                    if act_split:
                        for j in range(2):
                            nc.scalar.activation(
                                scr[:, j * hb : (j + 1) * hb],
                                s2[:, j * hb : (j + 1) * hb],
                                mybir.ActivationFunctionType.Abs,
                                accum_out=acc[:, 2 * bi + j : 2 * bi + j + 1],
                            )
                    else:
                        nc.scalar.activation(
                            scr[:, : 2 * hb],
                            s2[:, : 2 * hb],
                            mybir.ActivationFunctionType.Abs,
                            accum_out=acc[:, bi : bi + 1],
                        )
                    k += nb
                    bi += 1
            nc.sync.dma_start(out[:], acc[:])
    return nc


_NC = None


def _get_nc():
    global _NC
    if _NC is None:
        _NC = _hoist_excess_waits(_build())
    return _NC


def kernel(pred: np.ndarray, target: np.ndarray) -> np.ndarray:
    pred = np.ascontiguousarray(np.asarray(pred, dtype=np.float32))
    target = np.ascontiguousarray(np.asarray(target, dtype=np.float32))
    nc = _get_nc()
    vmat = make_vmat()
    in_maps = [
        {
            "pred": pred[i * B_LOC : (i + 1) * B_LOC],
            "target": target[i * B_LOC : (i + 1) * B_LOC],
            "vmat": vmat,
        }
        for i in range(N_CORES)
    ]
    trace = os.environ.get("DWT_KERNEL_TRACE") == "1"
    core_ids = list(range(N_CORES))
    try:
        res = run_bass_kernel_spmd(nc, in_maps, core_ids=core_ids, trace=trace)
    except ModuleNotFoundError:
        # axon NTFF profile hook unavailable in this environment
        res = run_bass_kernel_spmd(nc, in_maps, core_ids=core_ids, trace=False)
    if trace and res.exec_time_ns is not None:
        print(f"HW exec time: {res.exec_time_ns} ns")
    total = 0.0
    for r in res.results:
        total += float(r["partial"].astype(np.float64).sum())
    return np.float32(total / (2.0 * N_SUB))
